# revision 17
# baseline (speedup 1.0000x reference)
"""GroupedExpertNetwork (SwiGLU per-expert MLP) Trainium2 kernel.

Expert-parallel: 8 experts -> 8 NeuronCores, one expert per core.
Per core:  g = x @ gate; u = x @ inner; h = silu(g)*u; out = h @ outp
Shapes per expert: x [T=2048, D=2048], gate/inner [D, I=4096], outp [I, D].

Strategy:
- Host: transpose x -> xT [D, T], cast everything to bf16, and pre-pack all
  streamed tensors into tile-contiguous layouts so each device DMA is one
  contiguous block.
- Device: T-blocked with TB=1024 (2 strips). Per strip, compute hT [I, TB]
  on-chip (fully resident in SBUF, bf16), then the output matmuls accumulate
  over the full I=4096 in PSUM. All matmul free dims are 512 so bf16 runs at
  1 cycle/row on the PE and each PSUM tile is exactly one bank.
- TB=1024 (vs 512) halves HBM weight traffic: gate/inner/output weights are
  streamed twice total instead of four times (~120 MB vs ~227 MB per core),
  keeping the kernel compute-bound even under HBM bandwidth contention.
- Startup: the first matmul only needs xt piece 0 (2 MB) + the first small
  gate/inner chunk (1 MB with IC=128), and a short burst of warm-up matmuls
  on a zeroed scratch tile keeps the PE busy (and HAM un-throttled) while
  the first real tiles stream in.
"""

import numpy as np
import ml_dtypes

E, T, D, I = 8, 2048, 2048, 4096
P = 128
TB = 1024                # T strip size
NT = T // TB             # 2 strips
TH = 512                 # half-strip = matmul free dim (one PSUM bank)
NTH = TB // TH           # 2
IC = 128                 # I chunk for gate/inner weight streaming
NIC = I // IC            # 32
DC = 512                 # D chunk for output weight streaming
NDC = D // DC            # 4
KD = D // P              # 16 contraction chunks for gate/inner matmuls
KD2 = KD // 2            # xt pieces are split into two k-halves
KI = I // P              # 32 contraction chunks for output matmul
NWARM = 18               # warm-up matmuls issued while first DMAs land

_COMPILED = None


def _build_program():
    import concourse.mybir as mybir
    import concourse.tile as tile
    from concourse import bacc

    bf16 = mybir.dt.bfloat16
    f32 = mybir.dt.float32

    nc = bacc.Bacc(
        "TRN2",
        target_bir_lowering=False,
        debug=False,
        num_devices=E,
    )

    # Packed DRAM inputs (per core = one expert):
    # xt:  [NT, NTH, 2, P, KD2, TH]  xT tiles, d = (kh*KD2+kl)*128+p,
    #                                t = tb*TB+th*TH+c; split in two k-halves
    #                                so the first matmul's DMA is small
    # gw:  [NIC, P, KD, IC]      gate tiles
    # uw:  [NIC, P, KD, IC]      inner tiles
    # ow:  [NDC, P, KI, DC]      output-proj tiles
    xt_d = nc.dram_tensor("xt", (NT, NTH, 2, P, KD2, TH), bf16, kind="ExternalInput")
    gw_d = nc.dram_tensor("gw", (NIC, P, KD, IC), bf16, kind="ExternalInput")
    uw_d = nc.dram_tensor("uw", (NIC, P, KD, IC), bf16, kind="ExternalInput")
    ow_d = nc.dram_tensor("ow", (NDC, P, KI, DC), bf16, kind="ExternalInput")
    out_d = nc.dram_tensor("out", (T, D), bf16, kind="ExternalOutput")

    xt_ap = xt_d.ap()
    gw_ap = gw_d.ap()
    uw_ap = uw_d.ap()
    ow_ap = ow_d.ap()
    # out rows = tb*128 + p
    out_ap = out_d.ap().rearrange("(tb p) d -> tb p d", p=P)

    with tile.TileContext(nc) as tc:
        with (
            tc.tile_pool(name="xt", bufs=4) as xt_pool,
            tc.tile_pool(name="w", bufs=2) as w_pool,
            tc.tile_pool(name="ow", bufs=2) as ow_pool,
            tc.tile_pool(name="ht", bufs=1) as ht_pool,
            tc.tile_pool(name="tmp", bufs=3) as tmp_pool,
            tc.tile_pool(name="osb", bufs=3) as osb_pool,
            tc.tile_pool(name="warm", bufs=1) as warm_pool,
            tc.tile_pool(name="pg", bufs=2, space="PSUM") as pg_pool,
            tc.tile_pool(name="pu", bufs=2, space="PSUM") as pu_pool,
            tc.tile_pool(name="po", bufs=2, space="PSUM") as po_pool,
            tc.tile_pool(name="pw", bufs=1, space="PSUM") as pw_pool,
        ):
            # PE warm-up: matmuls on a zeroed scratch tile, no DMA deps, so
            # they run while the first real tiles stream in from HBM.
            warm = warm_pool.tile([P, TH], bf16, tag="warm")
            nc.vector.memset(warm[:], 0.0)
            pw = pw_pool.tile([P, TH], f32, tag="pw")
            for _ in range(NWARM):
                nc.tensor.matmul(pw[:], warm[:, 0:P], warm[:], start=True, stop=True)

            for tblk in range(NT):
                # The first matmul's working set is kept small and fed through
                # the scalar engine's HW-DGE ring (its preamble finishes ~2us
                # before the sync engine's, and its ring is otherwise empty),
                # in priority order: xa(th0) -> gw0 -> xb(th0) -> uw0. The
                # second half-strip rides the sync ring in parallel.
                xt00 = xt_pool.tile([P, KD2, TH], bf16, tag="xt")
                nc.sync.dma_start(xt00[:], xt_ap[tblk, 0, 0])
                xtp = [[xt00, None], [None, None]]

                ht = ht_pool.tile([P, KI, TB], bf16, tag="ht")

                for ic in range(NIC):
                    gw = w_pool.tile([P, KD, IC], bf16, tag="gw")
                    uw = w_pool.tile([P, KD, IC], bf16, tag="uw")
                    nc.sync.dma_start(gw[:], gw_ap[ic])
                    if ic == 0:
                        xt01 = xt_pool.tile([P, KD2, TH], bf16, tag="xt")
                        nc.sync.dma_start(xt01[:], xt_ap[tblk, 0, 1])
                        xtp[0][1] = xt01
                    nc.sync.dma_start(uw[:], uw_ap[ic])
                    if ic == 0:
                        for kh in range(2):
                            xt1 = xt_pool.tile([P, KD2, TH], bf16, tag="xt")
                            nc.sync.dma_start(xt1[:], xt_ap[tblk, 1, kh])
                            xtp[1][kh] = xt1

                    for th in range(NTH):
                        pg = pg_pool.tile([P, TH], f32, tag="pg")
                        for k in range(KD):
                            nc.tensor.matmul(
                                pg[:],
                                gw[:, k, :],
                                xtp[th][k // KD2][:, k % KD2, :],
                                start=(k == 0),
                                stop=(k == KD - 1),
                            )
                        pu = pu_pool.tile([P, TH], f32, tag="pu")
                        for k in range(KD):
                            nc.tensor.matmul(
                                pu[:],
                                uw[:, k, :],
                                xtp[th][k // KD2][:, k % KD2, :],
                                start=(k == 0),
                                stop=(k == KD - 1),
                            )
                        tmp = tmp_pool.tile([P, TH], f32, tag="tmp")
                        nc.scalar.activation(
                            tmp[:], pg[:], mybir.ActivationFunctionType.Silu
                        )
                        nc.vector.tensor_tensor(
                            ht[:, ic, th * TH:(th + 1) * TH],
                            tmp[:],
                            pu[:],
                            mybir.AluOpType.mult,
                        )

                for dc in range(NDC):
                    ow = ow_pool.tile([P, KI, DC], bf16, tag="ow")
                    nc.sync.dma_start(ow[:], ow_ap[dc])
                    for ti in range(TB // P):
                        po = po_pool.tile([P, DC], f32, tag="po")
                        for k in range(KI):
                            nc.tensor.matmul(
                                po[:],
                                ht[:, k, ti * P:(ti + 1) * P],
                                ow[:, k, :],
                                start=(k == 0),
                                stop=(k == KI - 1),
                            )
                        osb = osb_pool.tile([P, DC], bf16, tag="osb")
                        nc.vector.tensor_copy(osb[:], po[:])
                        nc.sync.dma_start(
                            out_ap[tblk * (TB // P) + ti, :, dc * DC:(dc + 1) * DC],
                            osb[:],
                        )

    nc.compile()
    return nc


def _get_program():
    global _COMPILED
    if _COMPILED is None:
        _COMPILED = _build_program()
    return _COMPILED


def _pack_inputs(x, gate_proj, inner_proj, output_proj):
    bf16 = ml_dtypes.bfloat16
    in_maps = []
    for e in range(E):
        # xT [D, T] -> [NT, NTH, 2, P, KD2, TH]; d = (kh*KD2+kl)*P + p
        xt = np.ascontiguousarray(x[e].T).astype(bf16)
        xt = xt.reshape(2, KD2, P, NT, NTH, TH).transpose(3, 4, 0, 2, 1, 5)
        xt = np.ascontiguousarray(xt)
        # gate [D, I] -> [NIC, P, KD, IC]
        gw = gate_proj[e].astype(bf16).reshape(KD, P, NIC, IC).transpose(2, 1, 0, 3)
        gw = np.ascontiguousarray(gw)
        uw = inner_proj[e].astype(bf16).reshape(KD, P, NIC, IC).transpose(2, 1, 0, 3)
        uw = np.ascontiguousarray(uw)
        # outp [I, D] -> [NDC, P, KI, DC]
        ow = output_proj[e].astype(bf16).reshape(KI, P, NDC, DC).transpose(2, 1, 0, 3)
        ow = np.ascontiguousarray(ow)
        in_maps.append({"xt": xt, "gw": gw, "uw": uw, "ow": ow})
    return in_maps


def kernel(x, gate_proj, inner_proj, output_proj, _trace=False, _trace_kwargs=None):
    from concourse import bass_utils

    nc = _get_program()
    in_maps = _pack_inputs(
        np.asarray(x), np.asarray(gate_proj), np.asarray(inner_proj),
        np.asarray(output_proj),
    )
    res = bass_utils.run_bass_kernel_spmd(
        nc,
        in_maps,
        core_ids=list(range(E)),
        trace=_trace,
        **(_trace_kwargs or {}),
    )
    out = np.stack(
        [np.asarray(res.results[e]["out"]).astype(np.float32) for e in range(E)]
    )
    if _trace:
        return out, res
    return out
